# revision 10
# baseline (speedup 1.0000x reference)
"""BinConv2d via 1D-horizontal Winograd F(2,3) on 8 TRN2 NeuronCores.

Strategy
--------
Data-parallel over batch: 32 images -> 4 per core; weights/BN replicated.

BN+binactive folds into a per-channel threshold U[c] (exact rational math,
same as the direct-conv baseline): device binarizes with ONE scalar-engine
activation  xb = Sign(x - U[c]) in {-1,+1} (fp16), padded with -1.

The 3x3 conv is computed with Winograd F(2,3) along the width axis only
(vertical taps stay direct, accumulating in PSUM):
    per ci-chunk the DVE forms 4 transform planes (exact small ints)
        H0 = d0-d2, H1 = d1+d2, H2 = d2-d1, H3 = d1-d3
    over column pairs (28 tiles); per output-row group (14 rows) and
    co-chunk, 4 PSUM planes M_j accumulate 6 matmuls each (3 kh x 2 cc)
    with host-transformed weights Gw = G @ [w0,w1,w2]; finally
        y_even = M0+M1+M2,  y_odd = M1-M2-M3
    via scalar-engine PSUM->SBUF fp16 eviction + 2x-mode DVE adds on flat
    [128,392] tiles.  Tensor-engine column count is 2/3 of direct conv
    (6 vs 9 MACs per output): ~125us of PE streaming vs 188us direct.

Every image streams in 4 row-chunks whose load steps (DMA -> binarize ->
H planes) are interleaved in program order with the previous image's conv
groups, so no engine queue ever head-of-line blocks on a far-future DMA
(that stall cost 13.6us of PE idle + HAM re-throttle in the v1 schedule).

Output leaves the device as fp16 y0/y1 parity planes; the host
de-interleaves and upcasts to fp32.
"""

from fractions import Fraction

import numpy as np

import concourse.bass as bass
import concourse.mybir as mybir
from concourse.bass_utils import run_bass_kernel_spmd
from concourse.tile import TileContext

N, C, H, W_ = 32, 256, 56, 56
NCORES = 8
IMGS = N // NCORES          # 4 images per core
KH = 3
GR = 14                     # output rows per group
NG = H // GR                # 4 groups
JT = W_ // 2                # 28 column-pair tiles
FREE = GR * JT              # 392
NB = 2 * 4 * KH             # 24 weight mats (cc, j, kh)
BN_EPS = np.float32(1e-4)

XB = [0, 15, 29, 43, 56]    # x-row chunk bounds
PB = [0, 16, 30, 44, 58]    # padded/H-row chunk bounds

_NC = None


def _legalize_waits(nc):
    """The TRN2 ISA takes ONE sync-wait per instruction, but Tile's wait
    assignment can attach several.  Split the extras into preceding
    same-engine NoOps, each carrying a single wait."""
    k = 0
    for fn in nc.m.functions:
        for blk in fn.blocks:
            new_insts = []
            for inst in blk.instructions:
                si = inst.sync_info
                waits = list(si.on_wait) if si and si.on_wait else []
                if len(waits) > 1:
                    for w in waits[:-1]:
                        nop = mybir.InstNoOp(name=f"waitsplit-{k}")
                        k += 1
                        nop.engine = inst.engine
                        nop.bass_nofuse = True
                        nop.sync_info = mybir.SyncInfo(on_wait=[w], on_update=[])
                        new_insts.append(nop)
                    inst.sync_info = mybir.SyncInfo(
                        on_wait=[waits[-1]],
                        on_update=list(si.on_update) if si.on_update else [])
                new_insts.append(inst)
            blk.instructions = new_insts


def _build_nc():
    nc = bass.Bass("TRN2")
    xs = nc.dram_tensor("xs", [IMGS, C, H, W_], mybir.dt.float32, kind="ExternalInput")
    # [p, coj, b, co128] weights ++ bit-packed -U thresholds at the tail
    cw = nc.dram_tensor("cw", [128, NB * C + 4], mybir.dt.float16,
                        kind="ExternalInput")
    # packed fp16 output: [img, coj, group, part, parity, row*jt]
    y = nc.dram_tensor("y", [IMGS, 2, NG, 128, 2 * FREE], mybir.dt.float16,
                       kind="ExternalOutput")

    with TileContext(nc) as tc:
        with (
            tc.tile_pool(name="const", bufs=1) as constp,
            tc.tile_pool(name="xin", bufs=4) as xinp,
            tc.tile_pool(name="pb", bufs=4) as pbp,
            tc.tile_pool(name="hx", bufs=4) as hp,
            tc.tile_pool(name="m", bufs=12) as mp,
            tc.tile_pool(name="ot", bufs=8) as otp,
            tc.tile_pool(name="ps", bufs=8, space="PSUM") as psp,
        ):
            # warm the PE clock on a dependency-free junk tile so real
            # matmuls (first ready ~13us in) start at K=8/8; the warm panel
            # borrows a PSUM pool buffer that recycles into the conv rotation
            junk = constp.tile([128, 448], mybir.dt.float16, tag="junk")
            nc.gpsimd.memset(junk[:], 0.25)
            wps = psp.tile([128, GR, JT], mybir.dt.float32, tag="ps")
            jr = junk[:, 0:FREE].rearrange("p (a b) -> p a b", a=GR)
            for i in range(20):
                nc.tensor.matmul(wps[:], lhsT=junk[:, 0:128], rhs=jr,
                                 start=True, stop=True)
            # touch the activation LUT early so the table load (~1.3us) is
            # off the first-binarize critical path
            jact = constp.tile([128, 8], mybir.dt.float16, tag="jact")
            nc.scalar.activation(out=jact[:], in_=junk[:, 0:8],
                                 func=mybir.ActivationFunctionType.Sign,
                                 bias=0.0, scale=1.0)

            cw_sb = constp.tile([128, NB * C + 4], mybir.dt.float16, tag="cw")
            # thresholds first (binarize gate), then per-j weight blocks
            nc.sync.dma_start(out=cw_sb[:, NB * C:NB * C + 4],
                              in_=cw[:, NB * C:NB * C + 4])
            w_sb = cw_sb[:, :NB * C].rearrange("p (j coj b c) -> p j coj b c",
                                               j=4, coj=2, b=6)
            uv_sb = cw_sb[:, NB * C:NB * C + 4].bitcast(mybir.dt.float32)

            def binarize(pb, xt, cc, ci):
                r0, r1 = XB[ci], XB[ci + 1]
                nc.scalar.activation(
                    out=pb[:, 1 + r0:1 + r1, 1:W_ + 1],
                    in_=xt[:, r0:r1, :],
                    func=mybir.ActivationFunctionType.Sign,
                    bias=uv_sb[:, cc:cc + 1],
                    scale=1.0,
                )

            def borders(pb):
                nc.vector.memset(pb[:, 0:H + 2:H + 1, :], -1.0)
                nc.vector.memset(pb[:, 1:H + 1, 0:W_ + 2:W_ + 1], -1.0)

            def hplanes(ht, pb, a, b):
                # planes 0-2 on the DVE, plane 3 on the (otherwise idle)
                # GpSimd engine
                nc.vector.tensor_sub(ht[:, 0, a:b, :],
                                     pb[:, a:b, 0:56:2], pb[:, a:b, 2:58:2])
                nc.vector.tensor_add(ht[:, 1, a:b, :],
                                     pb[:, a:b, 1:57:2], pb[:, a:b, 2:58:2])
                nc.vector.tensor_sub(ht[:, 2, a:b, :],
                                     pb[:, a:b, 2:58:2], pb[:, a:b, 1:57:2])
                nc.vector.tensor_sub(ht[:, 3, a:b, :],
                                     pb[:, a:b, 1:57:2], pb[:, a:b, 3:58:2])

            hts = [[None] * 2 for _ in range(IMGS)]
            state = [[None] * 2 for _ in range(IMGS)]

            def load_step(img, step, hgrain):
                ci, cc = divmod(step, 2)
                if ci == 0:
                    xt = xinp.tile([128, H, W_], mybir.dt.float32, tag="xin")
                    pb = pbp.tile([128, H + 2, W_ + 2], mybir.dt.float16, tag="pb")
                    ht = hp.tile([128, 4, H + 2, JT], mybir.dt.float16, tag="hx")
                    state[img][cc] = (xt, pb)
                    hts[img][cc] = ht
                    borders(pb)
                xt, pb = state[img][cc]
                r0, r1 = XB[ci], XB[ci + 1]
                nc.sync.dma_start(out=xt[:, r0:r1, :],
                                  in_=xs[img, cc * 128:(cc + 1) * 128, r0:r1, :])
                binarize(pb, xt, cc, ci)
                if hgrain == 1:          # per-chunk H (lowest latency, img0)
                    hplanes(hts[img][cc], pb, PB[ci], PB[ci + 1])
                elif ci in (1, 3):       # half-image H (fewer DVE ops)
                    hplanes(hts[img][cc], pb, PB[ci - 1], PB[ci + 1])

            # img0 loads upfront, with the coj0 weight half DMA'd after the
            # first chunk (first matmul gate) and the coj1 half after the
            # second
            # interleave the coj0 halves of the per-j weight blocks (0.2MB
            # each) with the x chunks on the DMA ring so no early matmul
            # gates on a bulk weight transfer; the coj1 halves stream last
            JBLK = 2 * 6 * 128
            HBLK = 6 * 128

            def cwdma(jj, coj):
                o = jj * JBLK + coj * HBLK
                nc.sync.dma_start(out=cw_sb[:, o:o + HBLK], in_=cw[:, o:o + HBLK])

            load_step(0, 0, 1)
            cwdma(0, 0)
            load_step(0, 1, 1)
            cwdma(1, 0)
            load_step(0, 2, 1)
            load_step(0, 3, 1)
            cwdma(2, 0)
            load_step(0, 4, 0)
            load_step(0, 5, 0)
            cwdma(3, 0)
            load_step(0, 6, 2)
            load_step(0, 7, 2)
            for jj in range(4):
                cwdma(jj, 1)

            def conv_rows(img, coj, g, r0, r1, ot, oto):
                # rows [r0, r1) of group g -> ot[:, :, oto:oto+(r1-r0)*JT]
                nr = r1 - r0
                ms = []
                for j in range(4):
                    ps = psp.tile([128, nr, JT], mybir.dt.float32, tag="ps")
                    for st in range(6):
                        cc, kh = divmod(st, KH)
                        rs = g * GR + r0 + kh
                        nc.tensor.matmul(
                            ps[:],
                            lhsT=w_sb[:, j, coj, cc * KH + kh, :],
                            rhs=hts[img][cc][:, j, rs:rs + nr, :],
                            start=(st == 0),
                            stop=(st == 5),
                        )
                    m = mp.tile([128, FREE], mybir.dt.float16, tag="m")
                    nc.scalar.copy(out=m[:, 0:nr * JT],
                                   in_=ps[:].rearrange("p a b -> p (a b)"))
                    ms.append(m)
                t0 = mp.tile([128, FREE], mybir.dt.float16, tag="m")
                t1 = mp.tile([128, FREE], mybir.dt.float16, tag="m")
                sl = slice(0, nr * JT)
                nc.vector.tensor_add(t0[:, sl], ms[0][:, sl], ms[1][:, sl])
                nc.vector.tensor_sub(t1[:, sl], ms[1][:, sl], ms[2][:, sl])
                nc.vector.tensor_add(ot[:, 0, oto:oto + nr * JT], t0[:, sl], ms[2][:, sl])
                nc.vector.tensor_sub(ot[:, 1, oto:oto + nr * JT], t1[:, sl], ms[3][:, sl])

            def conv_group(img, coj, g, split=False):
                ot = otp.tile([128, 2, FREE], mybir.dt.float16, tag="ot")
                if split:
                    half = GR // 2
                    conv_rows(img, coj, g, 0, half, ot, 0)
                    conv_rows(img, coj, g, half, GR, ot, half * JT)
                else:
                    conv_rows(img, coj, g, 0, GR, ot, 0)
                nc.sync.dma_start(out=y[img, coj, g],
                                  in_=ot[:].rearrange("p a b -> p (a b)"))

            # Load steps AFTER each group so the group's PSUM evacuations
            # sit ahead of the next image's binarize in the scalar FIFO.
            for img in range(IMGS):
                for it, (coj, g) in enumerate(
                        (cj, gg) for cj in range(2) for gg in range(NG)):
                    last = (img == IMGS - 1 and it >= 6)
                    conv_group(img, coj, g, split=last)
                    if img + 1 < IMGS:
                        load_step(img + 1, it, 2)
    return nc


def _get_nc():
    global _NC
    if _NC is None:
        _NC = _build_nc()
        _legalize_waits(_NC)
    return _NC


def _cr_rsqrt_f32(yv: np.float32) -> np.float32:
    """Correctly-rounded fp32 1/sqrt(y) (round-to-nearest-even) — bitwise
    identical to XLA's rsqrt on both the cpu and neuron backends."""
    fy = Fraction(float(yv))
    r0 = np.float32(1.0 / np.sqrt(float(yv)))
    cands = {float(r0)}
    lo = hi = r0
    for _ in range(2):
        lo = np.nextafter(lo, np.float32(-np.inf), dtype=np.float32)
        hi = np.nextafter(hi, np.float32(np.inf), dtype=np.float32)
        cands.update((float(lo), float(hi)))
    cands = sorted(cands)

    def gt(r):  # r > 1/sqrt(y)  <=>  r^2 * y > 1   (r > 0)
        return (Fraction(r) ** 2 * fy) > 1

    a = b = None
    for i in range(len(cands) - 1):
        if (not gt(cands[i])) and gt(cands[i + 1]):
            a, b = cands[i], cands[i + 1]
            break
    assert a is not None, "rsqrt bracket failure"
    m2 = Fraction(a + b) ** 2 * fy
    if m2 > 4:
        return np.float32(a)
    if m2 < 4:
        return np.float32(b)
    return np.float32(a) if (np.float32(a).view(np.int32) % 2 == 0) else np.float32(b)


def _thresholds(gamma, beta, running_mean, running_var) -> np.ndarray:
    """Per-channel U so that (x > U[c]) reproduces the reference's
    binarization decision bit-exactly (see the baseline kernel for the
    derivation)."""
    yv = (running_var + BN_EPS).astype(np.float32)
    inv = np.array([_cr_rsqrt_f32(v) for v in yv], dtype=np.float32)
    s = (gamma * inv).astype(np.float32)
    M = Fraction(1, 2) + Fraction(1, 2 ** 25)

    U = np.zeros(C, dtype=np.float32)
    for c in range(C):
        sc, bc, mc = s[c], beta[c], running_mean[c]
        assert sc > 0, "threshold fold assumes positive BN scale"
        fs, fb = Fraction(float(sc)), Fraction(float(bc))

        def dec(xv):
            t1 = np.float32(xv) - mc
            return Fraction(float(t1)) * fs + fb > M

        xv = np.float32(np.float64(mc) + (0.5 - np.float64(bc)) / np.float64(sc))
        guard = 0
        while dec(xv):
            xv = np.nextafter(xv, np.float32(-np.inf), dtype=np.float32)
            guard += 1
            assert guard < 10000, "threshold search diverged"
        nxt = np.nextafter(xv, np.float32(np.inf), dtype=np.float32)
        while not dec(nxt):
            xv = nxt
            nxt = np.nextafter(xv, np.float32(np.inf), dtype=np.float32)
            guard += 1
            assert guard < 10000, "threshold search diverged"
        U[c] = xv  # largest fp32 x that binarizes to -1:  device does x > U
    return U


def _prep_inputs(x, gamma, beta, running_mean, running_var, W):
    U = _thresholds(
        np.asarray(gamma, dtype=np.float32),
        np.asarray(beta, dtype=np.float32),
        np.asarray(running_mean, dtype=np.float32),
        np.asarray(running_var, dtype=np.float32),
    )
    negu = np.ascontiguousarray((-U).reshape(2, 128).T)  # [p, cc] fp32

    # Winograd-transformed weights in fp64 -> fp16:
    #   Gw[j] = row j of G @ [w0,w1,w2] per (co, ci)
    W64 = np.asarray(W, dtype=np.float64)
    g0, g1, g2 = W64[:, :, :, 0], W64[:, :, :, 1], W64[:, :, :, 2]  # [co,ci,kh]
    Gw = np.stack([g0, (g0 + g1 + g2) / 2, (g0 - g1 + g2) / 2, g2], axis=0)
    Gw16 = Gw.astype(np.float16)  # [4, co, ci, kh]

    wt = np.empty((128, 4, 2, 6, 128), dtype=np.float16)
    for j in range(4):
        for coj in range(2):
            for cc in range(2):
                for kh in range(KH):
                    wt[:, j, coj, cc * KH + kh, :] = Gw16[
                        j, coj * 128:(coj + 1) * 128,
                        cc * 128:(cc + 1) * 128, kh].T
    cw_dev = np.ascontiguousarray(
        np.concatenate([wt.reshape(128, NB * C), negu.view(np.float16)], axis=1))

    x = np.ascontiguousarray(np.asarray(x, dtype=np.float32))
    in_maps = [
        {"xs": x[i * IMGS:(i + 1) * IMGS], "cw": cw_dev}
        for i in range(NCORES)
    ]
    return in_maps


def _run(in_maps, trace=False, **kwargs):
    return run_bass_kernel_spmd(
        _get_nc(), in_maps, list(range(NCORES)), trace=trace, **kwargs)


def _gather(res):
    outs = []
    for i in range(NCORES):
        a = res.results[i]["y"].reshape(IMGS, 2, NG, 128, 2, GR, JT)
        # [img, coj, g, p, q, r, jt] -> [img, co, row, col]
        a = a.transpose(0, 1, 3, 2, 5, 6, 4).reshape(IMGS, C, H, W_)
        outs.append(a.astype(np.float32))
    return np.concatenate(outs, axis=0)


def kernel(x, gamma, beta, running_mean, running_var, W):
    in_maps = _prep_inputs(x, gamma, beta, running_mean, running_var, W)
    res = _run(in_maps)
    return _gather(res)


# revision 11
# speedup vs baseline: 1.0205x; 1.0205x over previous
"""BinConv2d via 1D-horizontal Winograd F(2,3) on 8 TRN2 NeuronCores.

Strategy
--------
Data-parallel over batch: 32 images -> 4 per core; weights/BN replicated.

BN+binactive folds into a per-channel threshold U[c] (exact rational math,
same as the direct-conv baseline): device binarizes with ONE scalar-engine
activation  xb = Sign(x - U[c]) in {-1,+1} (fp16), padded with -1.

The 3x3 conv is computed with Winograd F(2,3) along the width axis only
(vertical taps stay direct, accumulating in PSUM):
    per ci-chunk the DVE forms 4 transform planes (exact small ints)
        H0 = d0-d2, H1 = d1+d2, H2 = d2-d1, H3 = d1-d3
    over column pairs (28 tiles); per output-row group (14 rows) and
    co-chunk, 4 PSUM planes M_j accumulate 6 matmuls each (3 kh x 2 cc)
    with host-transformed weights Gw = G @ [w0,w1,w2]; finally
        y_even = M0+M1+M2,  y_odd = M1-M2-M3
    via scalar-engine PSUM->SBUF fp16 eviction + 2x-mode DVE adds on flat
    [128,392] tiles.  Tensor-engine column count is 2/3 of direct conv
    (6 vs 9 MACs per output): ~125us of PE streaming vs 188us direct.

Every image streams in 4 row-chunks whose load steps (DMA -> binarize ->
H planes) are interleaved in program order with the previous image's conv
groups, so no engine queue ever head-of-line blocks on a far-future DMA
(that stall cost 13.6us of PE idle + HAM re-throttle in the v1 schedule).

Output leaves the device as fp16 y0/y1 parity planes; the host
de-interleaves and upcasts to fp32.
"""

from fractions import Fraction

import numpy as np

import concourse.bass as bass
import concourse.mybir as mybir
from concourse.bass_utils import run_bass_kernel_spmd
from concourse.tile import TileContext

N, C, H, W_ = 32, 256, 56, 56
NCORES = 8
IMGS = N // NCORES          # 4 images per core
KH = 3
GR = 14                     # output rows per group
NG = H // GR                # 4 groups
JT = W_ // 2                # 28 column-pair tiles
FREE = GR * JT              # 392
NB = 2 * 4 * KH             # 24 weight mats (cc, j, kh)
BN_EPS = np.float32(1e-4)

XB = [0, 15, 29, 43, 56]    # x-row chunk bounds
PB = [0, 16, 30, 44, 58]    # padded/H-row chunk bounds

_NC = None


def _legalize_waits(nc):
    """The TRN2 ISA takes ONE sync-wait per instruction, but Tile's wait
    assignment can attach several.  Split the extras into preceding
    same-engine NoOps, each carrying a single wait."""
    k = 0
    for fn in nc.m.functions:
        for blk in fn.blocks:
            new_insts = []
            for inst in blk.instructions:
                si = inst.sync_info
                waits = list(si.on_wait) if si and si.on_wait else []
                if len(waits) > 1:
                    for w in waits[:-1]:
                        nop = mybir.InstNoOp(name=f"waitsplit-{k}")
                        k += 1
                        nop.engine = inst.engine
                        nop.bass_nofuse = True
                        nop.sync_info = mybir.SyncInfo(on_wait=[w], on_update=[])
                        new_insts.append(nop)
                    inst.sync_info = mybir.SyncInfo(
                        on_wait=[waits[-1]],
                        on_update=list(si.on_update) if si.on_update else [])
                new_insts.append(inst)
            blk.instructions = new_insts


def _build_nc():
    nc = bass.Bass("TRN2")
    xs = nc.dram_tensor("xs", [IMGS, C, H, W_], mybir.dt.float32, kind="ExternalInput")
    # [p, coj, b, co128] weights ++ bit-packed -U thresholds at the tail
    cw = nc.dram_tensor("cw", [128, NB * C + 4], mybir.dt.float16,
                        kind="ExternalInput")
    # packed fp16 output: [img, coj, group, part, parity, row*jt]
    y = nc.dram_tensor("y", [IMGS, 2, NG, 128, 2 * FREE], mybir.dt.float16,
                       kind="ExternalOutput")

    with TileContext(nc) as tc:
        with (
            tc.tile_pool(name="const", bufs=1) as constp,
            tc.tile_pool(name="xin", bufs=4) as xinp,
            tc.tile_pool(name="pb", bufs=4) as pbp,
            tc.tile_pool(name="hx", bufs=4) as hp,
            tc.tile_pool(name="m", bufs=12) as mp,
            tc.tile_pool(name="ot", bufs=8) as otp,
            tc.tile_pool(name="ps", bufs=8, space="PSUM") as psp,
        ):
            # warm the PE clock on a dependency-free junk tile so real
            # matmuls (first ready ~13us in) start at K=8/8; the warm panel
            # borrows a PSUM pool buffer that recycles into the conv rotation
            junk = constp.tile([128, 448], mybir.dt.float16, tag="junk")
            nc.gpsimd.memset(junk[:], 0.25)
            wps = psp.tile([128, GR, JT], mybir.dt.float32, tag="ps")
            jr = junk[:, 0:FREE].rearrange("p (a b) -> p a b", a=GR)
            for i in range(20):
                nc.tensor.matmul(wps[:], lhsT=junk[:, 0:128], rhs=jr,
                                 start=True, stop=True)
            # touch the activation LUT early so the table load (~1.3us) is
            # off the first-binarize critical path
            jact = constp.tile([128, 8], mybir.dt.float16, tag="jact")
            nc.scalar.activation(out=jact[:], in_=junk[:, 0:8],
                                 func=mybir.ActivationFunctionType.Sign,
                                 bias=0.0, scale=1.0)

            cw_sb = constp.tile([128, NB * C + 4], mybir.dt.float16, tag="cw")
            # thresholds first (binarize gate), then per-j weight blocks
            nc.sync.dma_start(out=cw_sb[:, NB * C:NB * C + 4],
                              in_=cw[:, NB * C:NB * C + 4])
            w_sb = cw_sb[:, :NB * C].rearrange("p (j coj b c) -> p j coj b c",
                                               j=4, coj=2, b=6)
            uv_sb = cw_sb[:, NB * C:NB * C + 4].bitcast(mybir.dt.float32)

            def binarize(pb, xt, cc, ci):
                r0, r1 = XB[ci], XB[ci + 1]
                nc.scalar.activation(
                    out=pb[:, 1 + r0:1 + r1, 1:W_ + 1],
                    in_=xt[:, r0:r1, :],
                    func=mybir.ActivationFunctionType.Sign,
                    bias=uv_sb[:, cc:cc + 1],
                    scale=1.0,
                )

            def borders(pb):
                nc.vector.memset(pb[:, 0:H + 2:H + 1, :], -1.0)
                nc.vector.memset(pb[:, 1:H + 1, 0:W_ + 2:W_ + 1], -1.0)

            def hplanes(ht, pb, a, b):
                # planes 0-2 on the DVE, plane 3 on the (otherwise idle)
                # GpSimd engine
                nc.vector.tensor_sub(ht[:, 0, a:b, :],
                                     pb[:, a:b, 0:56:2], pb[:, a:b, 2:58:2])
                nc.vector.tensor_add(ht[:, 1, a:b, :],
                                     pb[:, a:b, 1:57:2], pb[:, a:b, 2:58:2])
                nc.vector.tensor_sub(ht[:, 2, a:b, :],
                                     pb[:, a:b, 2:58:2], pb[:, a:b, 1:57:2])
                nc.vector.tensor_sub(ht[:, 3, a:b, :],
                                     pb[:, a:b, 1:57:2], pb[:, a:b, 3:58:2])

            hts = [[None] * 2 for _ in range(IMGS)]
            state = [[None] * 2 for _ in range(IMGS)]

            def load_step(img, step, hgrain):
                ci, cc = divmod(step, 2)
                if ci == 0:
                    xt = xinp.tile([128, H, W_], mybir.dt.float32, tag="xin")
                    pb = pbp.tile([128, H + 2, W_ + 2], mybir.dt.float16, tag="pb")
                    ht = hp.tile([128, 4, H + 2, JT], mybir.dt.float16, tag="hx")
                    state[img][cc] = (xt, pb)
                    hts[img][cc] = ht
                    borders(pb)
                xt, pb = state[img][cc]
                r0, r1 = XB[ci], XB[ci + 1]
                nc.sync.dma_start(out=xt[:, r0:r1, :],
                                  in_=xs[img, cc * 128:(cc + 1) * 128, r0:r1, :])
                binarize(pb, xt, cc, ci)
                if hgrain == 1:          # per-chunk H (lowest latency, img0)
                    hplanes(hts[img][cc], pb, PB[ci], PB[ci + 1])
                elif ci in (1, 3):       # half-image H (fewer DVE ops)
                    hplanes(hts[img][cc], pb, PB[ci - 1], PB[ci + 1])

            # img0 loads upfront, with the coj0 weight half DMA'd after the
            # first chunk (first matmul gate) and the coj1 half after the
            # second
            # interleave the coj0 halves of the per-j weight blocks (0.2MB
            # each) with the x chunks on the DMA ring so no early matmul
            # gates on a bulk weight transfer; the coj1 halves stream last
            JBLK = 2 * 6 * 128
            HBLK = 6 * 128

            def cwdma(jj, coj):
                o = jj * JBLK + coj * HBLK
                nc.sync.dma_start(out=cw_sb[:, o:o + HBLK], in_=cw[:, o:o + HBLK])

            load_step(0, 0, 1)
            cwdma(0, 0)
            load_step(0, 1, 1)
            cwdma(1, 0)
            load_step(0, 2, 1)
            load_step(0, 3, 1)
            cwdma(2, 0)
            load_step(0, 4, 1)
            load_step(0, 5, 1)
            cwdma(3, 0)
            load_step(0, 6, 1)
            load_step(0, 7, 1)
            for jj in range(4):
                cwdma(jj, 1)

            def conv_rows(img, coj, g, r0, r1, ot, oto):
                # rows [r0, r1) of group g -> ot[:, :, oto:oto+(r1-r0)*JT]
                nr = r1 - r0
                ms = []
                for j in range(4):
                    ps = psp.tile([128, nr, JT], mybir.dt.float32, tag="ps")
                    for st in range(6):
                        cc, kh = divmod(st, KH)
                        rs = g * GR + r0 + kh
                        nc.tensor.matmul(
                            ps[:],
                            lhsT=w_sb[:, j, coj, cc * KH + kh, :],
                            rhs=hts[img][cc][:, j, rs:rs + nr, :],
                            start=(st == 0),
                            stop=(st == 5),
                        )
                    m = mp.tile([128, FREE], mybir.dt.float16, tag="m")
                    nc.scalar.copy(out=m[:, 0:nr * JT],
                                   in_=ps[:].rearrange("p a b -> p (a b)"))
                    ms.append(m)
                t0 = mp.tile([128, FREE], mybir.dt.float16, tag="m")
                t1 = mp.tile([128, FREE], mybir.dt.float16, tag="m")
                sl = slice(0, nr * JT)
                nc.vector.tensor_add(t0[:, sl], ms[0][:, sl], ms[1][:, sl])
                nc.vector.tensor_sub(t1[:, sl], ms[1][:, sl], ms[2][:, sl])
                nc.vector.tensor_add(ot[:, 0, oto:oto + nr * JT], t0[:, sl], ms[2][:, sl])
                nc.vector.tensor_sub(ot[:, 1, oto:oto + nr * JT], t1[:, sl], ms[3][:, sl])

            def conv_group(img, coj, g, split=False):
                ot = otp.tile([128, 2, FREE], mybir.dt.float16, tag="ot")
                if split:
                    half = GR // 2
                    conv_rows(img, coj, g, 0, half, ot, 0)
                    conv_rows(img, coj, g, half, GR, ot, half * JT)
                else:
                    conv_rows(img, coj, g, 0, GR, ot, 0)
                nc.sync.dma_start(out=y[img, coj, g],
                                  in_=ot[:].rearrange("p a b -> p (a b)"))

            # Load steps AFTER each group so the group's PSUM evacuations
            # sit ahead of the next image's binarize in the scalar FIFO.
            for img in range(IMGS):
                for it, (coj, g) in enumerate(
                        (cj, gg) for cj in range(2) for gg in range(NG)):
                    last = (img == IMGS - 1 and it >= 6)
                    conv_group(img, coj, g, split=last)
                    if img + 1 < IMGS:
                        load_step(img + 1, it, 2)
    return nc


def _get_nc():
    global _NC
    if _NC is None:
        _NC = _build_nc()
        _legalize_waits(_NC)
    return _NC


def _cr_rsqrt_f32(yv: np.float32) -> np.float32:
    """Correctly-rounded fp32 1/sqrt(y) (round-to-nearest-even) — bitwise
    identical to XLA's rsqrt on both the cpu and neuron backends."""
    fy = Fraction(float(yv))
    r0 = np.float32(1.0 / np.sqrt(float(yv)))
    cands = {float(r0)}
    lo = hi = r0
    for _ in range(2):
        lo = np.nextafter(lo, np.float32(-np.inf), dtype=np.float32)
        hi = np.nextafter(hi, np.float32(np.inf), dtype=np.float32)
        cands.update((float(lo), float(hi)))
    cands = sorted(cands)

    def gt(r):  # r > 1/sqrt(y)  <=>  r^2 * y > 1   (r > 0)
        return (Fraction(r) ** 2 * fy) > 1

    a = b = None
    for i in range(len(cands) - 1):
        if (not gt(cands[i])) and gt(cands[i + 1]):
            a, b = cands[i], cands[i + 1]
            break
    assert a is not None, "rsqrt bracket failure"
    m2 = Fraction(a + b) ** 2 * fy
    if m2 > 4:
        return np.float32(a)
    if m2 < 4:
        return np.float32(b)
    return np.float32(a) if (np.float32(a).view(np.int32) % 2 == 0) else np.float32(b)


def _thresholds(gamma, beta, running_mean, running_var) -> np.ndarray:
    """Per-channel U so that (x > U[c]) reproduces the reference's
    binarization decision bit-exactly (see the baseline kernel for the
    derivation)."""
    yv = (running_var + BN_EPS).astype(np.float32)
    inv = np.array([_cr_rsqrt_f32(v) for v in yv], dtype=np.float32)
    s = (gamma * inv).astype(np.float32)
    M = Fraction(1, 2) + Fraction(1, 2 ** 25)

    U = np.zeros(C, dtype=np.float32)
    for c in range(C):
        sc, bc, mc = s[c], beta[c], running_mean[c]
        assert sc > 0, "threshold fold assumes positive BN scale"
        fs, fb = Fraction(float(sc)), Fraction(float(bc))

        def dec(xv):
            t1 = np.float32(xv) - mc
            return Fraction(float(t1)) * fs + fb > M

        xv = np.float32(np.float64(mc) + (0.5 - np.float64(bc)) / np.float64(sc))
        guard = 0
        while dec(xv):
            xv = np.nextafter(xv, np.float32(-np.inf), dtype=np.float32)
            guard += 1
            assert guard < 10000, "threshold search diverged"
        nxt = np.nextafter(xv, np.float32(np.inf), dtype=np.float32)
        while not dec(nxt):
            xv = nxt
            nxt = np.nextafter(xv, np.float32(np.inf), dtype=np.float32)
            guard += 1
            assert guard < 10000, "threshold search diverged"
        U[c] = xv  # largest fp32 x that binarizes to -1:  device does x > U
    return U


def _prep_inputs(x, gamma, beta, running_mean, running_var, W):
    U = _thresholds(
        np.asarray(gamma, dtype=np.float32),
        np.asarray(beta, dtype=np.float32),
        np.asarray(running_mean, dtype=np.float32),
        np.asarray(running_var, dtype=np.float32),
    )
    negu = np.ascontiguousarray((-U).reshape(2, 128).T)  # [p, cc] fp32

    # Winograd-transformed weights in fp64 -> fp16:
    #   Gw[j] = row j of G @ [w0,w1,w2] per (co, ci)
    W64 = np.asarray(W, dtype=np.float64)
    g0, g1, g2 = W64[:, :, :, 0], W64[:, :, :, 1], W64[:, :, :, 2]  # [co,ci,kh]
    Gw = np.stack([g0, (g0 + g1 + g2) / 2, (g0 - g1 + g2) / 2, g2], axis=0)
    Gw16 = Gw.astype(np.float16)  # [4, co, ci, kh]

    wt = np.empty((128, 4, 2, 6, 128), dtype=np.float16)
    for j in range(4):
        for coj in range(2):
            for cc in range(2):
                for kh in range(KH):
                    wt[:, j, coj, cc * KH + kh, :] = Gw16[
                        j, coj * 128:(coj + 1) * 128,
                        cc * 128:(cc + 1) * 128, kh].T
    cw_dev = np.ascontiguousarray(
        np.concatenate([wt.reshape(128, NB * C), negu.view(np.float16)], axis=1))

    x = np.ascontiguousarray(np.asarray(x, dtype=np.float32))
    in_maps = [
        {"xs": x[i * IMGS:(i + 1) * IMGS], "cw": cw_dev}
        for i in range(NCORES)
    ]
    return in_maps


def _run(in_maps, trace=False, **kwargs):
    return run_bass_kernel_spmd(
        _get_nc(), in_maps, list(range(NCORES)), trace=trace, **kwargs)


def _gather(res):
    outs = []
    for i in range(NCORES):
        a = res.results[i]["y"].reshape(IMGS, 2, NG, 128, 2, GR, JT)
        # [img, coj, g, p, q, r, jt] -> [img, co, row, col]
        a = a.transpose(0, 1, 3, 2, 5, 6, 4).reshape(IMGS, C, H, W_)
        outs.append(a.astype(np.float32))
    return np.concatenate(outs, axis=0)


def kernel(x, gamma, beta, running_mean, running_var, W):
    in_maps = _prep_inputs(x, gamma, beta, running_mean, running_var, W)
    res = _run(in_maps)
    return _gather(res)
